# revision 13
# baseline (speedup 1.0000x reference)
"""Trainium2 Bass kernel for nn_AffineExponential.

Computes, for each sample b:
    y_b   = expm(t_b * W) @ x_b + t_b * bias
    ljd_b = t_b * diag(W)

Key identity: expm(t W) x = sum_k (t^k / k!) W^k x, so instead of per-sample
matrix exponentials we run one shared chain of [128, B] matmuls with a scaled
recurrence  U_0 = X^T,  U_{k+1} = (W @ U_k) * t / (k+1)  and  y^T = sum_k U_k.
The per-column (per-sample) t scaling fuses into a single scalar_tensor_tensor
op per chain step; the sum over k accumulates in PSUM via identity-stationary
matmuls (start=False), so no vector/gpsimd adds are needed. K=11 terms reaches
the fp32 floor (spectral radius of W ~1.08, t in [0,1); term k <= 1.08^k/k!).

Sharding: pure data-parallel over the batch dim, 8 cores x 512 samples.
weight/bias replicated. All dims hardcoded per the harness contract.
"""

import sys
from contextlib import ExitStack

import numpy as np

for _p in ("/opt/trn_rl_repo", "/root/.axon_site/_ro/trn_rl_repo"):
    if _p not in sys.path:
        sys.path.append(_p)


def _ensure_ntff_hook_module():
    """The agent image's antenv lacks axon_hooks; provide it so
    run_bass_kernel_spmd's trace=True path can profile. No-op if present."""
    import types
    try:
        import antenv.axon_hooks  # noqa: F401
        return
    except ImportError:
        pass
    mod = types.ModuleType("antenv.axon_hooks")
    _state = {"hook": None}
    mod.set_axon_ntff_profile_hook = lambda h: _state.__setitem__("hook", h)
    mod.get_axon_ntff_profile_hook = lambda: _state["hook"]
    sys.modules["antenv.axon_hooks"] = mod
    try:
        from trn_agent_boot.trn_boot import _ntff_profile_via_ctypes
        mod.set_axon_ntff_profile_hook(
            _ntff_profile_via_ctypes("/opt/axon/libaxon_pjrt.so"))
    except Exception:
        pass


_ensure_ntff_hook_module()

import concourse.bass as bass
import concourse.tile as tile
from concourse import mybir
from concourse.bass_utils import run_bass_kernel_spmd

B, D = 4096, 128
N_CORES = 8
B_LOC = B // N_CORES  # 512
K_TERMS = 11  # terms 0..10; term k magnitude <= (t*rho)^k/k!, rho ~ 1.08
F32 = mybir.dt.float32
MULT = mybir.AluOpType.mult


def _hoist_waits(nc: bass.Bass) -> int:
    """Move semaphore waits off instructions onto standalone EventSemaphore
    instructions. This walrus build rejects any wait attached to a Matmult
    (S3_LW struct) and allows at most one elsewhere ("Too many sync wait
    commands"); a preceding same-engine wait instruction is equivalent."""
    n = 0
    for f in nc.m.functions:
        for blk in f.blocks:
            il = blk.instructions
            i = 0
            while i < len(il):
                ins = il[i]
                si = ins.sync_info
                if si is None or not si.on_wait:
                    i += 1
                    continue
                keep = 0 if ins.__class__.__name__ in ("InstMatmult", "InstMatmultMx") else 1
                waits = list(si.on_wait)
                if len(waits) <= keep:
                    i += 1
                    continue
                hoisted = waits[: len(waits) - keep]
                si.on_wait = waits[len(waits) - keep:]
                for w in hoisted:
                    wi = mybir.InstEventSemaphore(
                        name=f"W-hoist-{n}", engine=ins.engine, ins=[], outs=[])
                    wi.sync_info = type(si)(on_wait=[w], on_update=[])
                    il.insert(i, wi)
                    n += 1
                    i += 1
                i += 1
    return n


def _build_program(hoist: bool = True) -> bass.Bass:
    nc = bass.Bass("TRN2", target_bir_lowering=False, debug=False,
                   enable_asserts=False, num_devices=N_CORES,
                   enable_partition_id=False)

    x_d = nc.dram_tensor("x", [B_LOC, D], F32, kind="ExternalInput").ap()
    t_d = nc.dram_tensor("t", [B_LOC, 1], F32, kind="ExternalInput").ap()
    w_d = nc.dram_tensor("w", [D, D], F32, kind="ExternalInput").ap()
    b_d = nc.dram_tensor("b", [1, D], F32, kind="ExternalInput").ap()
    # host-supplied constants: [:, 0:128] = identity, [:, 128:256] = ones
    c_d = nc.dram_tensor("consts", [D, 2 * D], F32, kind="ExternalInput").ap()
    y_d = nc.dram_tensor("y", [B_LOC, D], F32, kind="ExternalOutput").ap()
    ljd_d = nc.dram_tensor("ljd", [B_LOC, D], F32, kind="ExternalOutput").ap()

    NT = B_LOC // D  # 4 batch tiles of 128
    HALF = B_LOC // 2  # 256: chain runs as two independent column-halves

    with tile.TileContext(nc) as tc, ExitStack() as ctx:
        const = ctx.enter_context(tc.tile_pool(name="const", bufs=1))
        upool = ctx.enter_context(tc.tile_pool(name="u", bufs=6))
        obuf = ctx.enter_context(tc.tile_pool(name="obuf", bufs=2))
        ps_sm = ctx.enter_context(tc.tile_pool(name="ps_sm", bufs=2, space="PSUM"))
        ps_t = ctx.enter_context(tc.tile_pool(name="ps_t", bufs=1, space="PSUM"))
        ps_y = ctx.enter_context(tc.tile_pool(name="ps_y", bufs=1, space="PSUM"))
        ps_chain = ctx.enter_context(tc.tile_pool(name="ps_chain", bufs=2, space="PSUM"))
        ps_out = ctx.enter_context(tc.tile_pool(name="ps_out", bufs=2, space="PSUM"))

        # ---- loads ----
        consts = const.tile([D, 2 * D], F32, tag="consts")
        nc.sync.dma_start(consts, c_d)
        ident = consts[:, 0:D]
        ones_col = consts[:, D:D + 1]
        ones_row = consts[0:1, D:2 * D]

        x_bm = const.tile([D, NT, D], F32, tag="x_bm")
        nc.sync.dma_start(x_bm, x_d.rearrange("(m p) i -> p m i", p=D))
        w_sb = const.tile([D, D], F32, tag="w_sb")
        nc.sync.dma_start(w_sb, w_d)
        t_row = const.tile([1, B_LOC], F32, tag="t_row")
        nc.sync.dma_start(t_row, t_d.rearrange("a b -> b a"))
        bias_row = const.tile([1, D], F32, tag="bias_row")
        nc.sync.dma_start(bias_row, b_d)

        # ---- layout transposes: XT = x^T (feature-major), WT = W^T ----
        xt = const.tile([D, B_LOC], F32, tag="xt")
        for m in range(NT):
            ps = ps_sm.tile([D, D], F32, tag="ps_sm")
            nc.tensor.transpose(ps, x_bm[:, m, :], ident)
            nc.scalar.copy(xt[:, bass.ts(m, D)], ps)
        wt = const.tile([D, D], F32, tag="wt")
        ps = ps_sm.tile([D, D], F32, tag="ps_sm")
        nc.tensor.transpose(ps, w_sb, ident)
        nc.scalar.copy(wt, ps)

        # ---- T_rep[i, b] = t_b (broadcast across partitions, rank-1 matmul)
        t_rep = const.tile([D, B_LOC], F32, tag="t_rep")
        psT = ps_t.tile([D, B_LOC], F32, tag="psT")
        nc.tensor.matmul(psT, ones_row, t_row)
        nc.scalar.copy(t_rep, psT)

        # ---- diag(W) as a row: ones_col^T @ (W .* I) ----
        wi = const.tile([D, D], F32, tag="wi")
        nc.gpsimd.tensor_mul(wi, w_sb, ident)
        ps = ps_sm.tile([D, D], F32, tag="ps_sm")
        nc.tensor.matmul(ps[0:1, :], ones_col, wi)
        diag_row = const.tile([1, D], F32, tag="diag_row")
        nc.scalar.copy(diag_row, ps[0:1, :])

        # ---- ljd = t x diag(W), early: fills PE while the chain waits ----
        lo_all = const.tile([D, NT, D], F32, tag="lo_all")
        for m in range(NT):
            psl = ps_out.tile([D, D], F32, tag="ps_out")
            nc.tensor.matmul(psl, t_row[0:1, bass.ts(m, D)], diag_row)
            nc.scalar.copy(lo_all[:, m, :], psl)
        nc.sync.dma_start(ljd_d.rearrange("(m p) i -> p m i", p=D), lo_all)

        # ---- Taylor chain. Y accumulates in PSUM via identity matmuls. ----
        psY = ps_y.tile([D, B_LOC], F32, tag="psY")
        cur = [xt[:, 0:HALF], xt[:, HALF:B_LOC]]
        # single full-width init: start=True clears has_written bank-wide, so
        # it must happen exactly once for this bank
        nc.tensor.matmul(psY, ident, xt, start=True, stop=False,
                         skip_group_check=True)
        for k in range(1, K_TERMS):
            last = k == K_TERMS - 1
            for h in range(2):
                psc = ps_chain.tile([D, HALF], F32, tag="ps_chain")
                nc.tensor.matmul(psc, wt, cur[h])
                u_next = upool.tile([D, HALF], F32, tag="u")
                nc.vector.scalar_tensor_tensor(out=u_next, in0=psc,
                                               scalar=float(1.0 / k),
                                               in1=t_rep[:, bass.ts(h, HALF)],
                                               op0=MULT, op1=MULT)
                nc.tensor.matmul(psY[:, bass.ts(h, HALF)], ident, u_next,
                                 start=False, stop=last, skip_group_check=True)
                cur[h] = u_next[:]

        y_fm = const.tile([D, B_LOC], F32, tag="y_fm")
        nc.scalar.copy(y_fm, psY)

        # ---- transpose Y to batch-major, accumulating bias x t in PSUM ----
        yo_all = const.tile([D, NT, D], F32, tag="yo_all")
        for m in range(NT):
            ps = ps_out.tile([D, D], F32, tag="ps_out")
            nc.tensor.transpose(ps, y_fm[:, bass.ts(m, D)], ident)
            nc.tensor.matmul(ps, t_row[0:1, bass.ts(m, D)], bias_row,
                             start=False, stop=True, skip_group_check=True)
            nc.scalar.copy(yo_all[:, m, :], ps)
        nc.sync.dma_start(y_d.rearrange("(m p) i -> p m i", p=D), yo_all)

    if hoist:
        _hoist_waits(nc)
    return nc


_CACHE: dict = {}


def _consts_np() -> np.ndarray:
    c = np.zeros((D, 2 * D), dtype=np.float32)
    c[:, :D] = np.eye(D, dtype=np.float32)
    c[:, D:] = 1.0
    return c


def _run(x, t, weight, bias, trace=False, **trace_kw):
    if "nc" not in _CACHE:
        _CACHE["nc"] = _build_program()
    nc = _CACHE["nc"]
    x = np.ascontiguousarray(x, dtype=np.float32)
    t = np.ascontiguousarray(t, dtype=np.float32)
    w = np.ascontiguousarray(weight, dtype=np.float32)
    b = np.ascontiguousarray(bias, dtype=np.float32).reshape(1, D)
    consts = _consts_np()
    in_maps = [
        {"x": x[i * B_LOC:(i + 1) * B_LOC], "t": t[i * B_LOC:(i + 1) * B_LOC],
         "w": w, "b": b, "consts": consts}
        for i in range(N_CORES)
    ]
    res = run_bass_kernel_spmd(nc, in_maps, list(range(N_CORES)),
                               trace=trace, **trace_kw)
    y = np.concatenate([res.results[i]["y"] for i in range(N_CORES)], axis=0)
    ljd = np.concatenate([res.results[i]["ljd"] for i in range(N_CORES)], axis=0)
    return (y, ljd), res


def kernel(x, t, weight, bias):
    (y, ljd), _ = _run(x, t, weight, bias, trace=False)
    return y, ljd


# revision 15
# speedup vs baseline: 1.2102x; 1.2102x over previous
"""Trainium2 Bass kernel for nn_AffineExponential.

Computes, for each sample b:
    y_b   = expm(t_b * W) @ x_b + t_b * bias
    ljd_b = t_b * diag(W)

Key identity: expm(t W) x = sum_k (t^k / k!) W^k x, so instead of per-sample
matrix exponentials we run one shared chain of [128, B] matmuls with a scaled
recurrence  U_0 = X^T,  U_{k+1} = (W @ U_k) * t / (k+1)  and  y^T = sum_k U_k.
The per-column (per-sample) t scaling fuses into a single scalar_tensor_tensor
op per chain step on the vector engine; the running sum lives in SBUF with the
adds split between the vector and gpsimd engines (one batch-half each). K=10
terms reaches the fp32 floor (spectral radius of W ~1.08, t in [0,1); term k
magnitude <= 1.08^k/k!).

Sharding: pure data-parallel over the batch dim, 8 cores x 512 samples.
weight/bias replicated. All dims hardcoded per the harness contract.
"""

import sys
from contextlib import ExitStack

import numpy as np

for _p in ("/opt/trn_rl_repo", "/root/.axon_site/_ro/trn_rl_repo"):
    if _p not in sys.path:
        sys.path.append(_p)


def _ensure_ntff_hook_module():
    """The agent image's antenv lacks axon_hooks; provide it so
    run_bass_kernel_spmd's trace=True path can profile. No-op if present."""
    import types
    try:
        import antenv.axon_hooks  # noqa: F401
        return
    except ImportError:
        pass
    mod = types.ModuleType("antenv.axon_hooks")
    _state = {"hook": None}
    mod.set_axon_ntff_profile_hook = lambda h: _state.__setitem__("hook", h)
    mod.get_axon_ntff_profile_hook = lambda: _state["hook"]
    sys.modules["antenv.axon_hooks"] = mod
    try:
        from trn_agent_boot.trn_boot import _ntff_profile_via_ctypes
        mod.set_axon_ntff_profile_hook(
            _ntff_profile_via_ctypes("/opt/axon/libaxon_pjrt.so"))
    except Exception:
        pass


_ensure_ntff_hook_module()

import concourse.bass as bass
import concourse.tile as tile
from concourse import mybir
from concourse.bass_utils import run_bass_kernel_spmd

B, D = 4096, 128
N_CORES = 8
B_LOC = B // N_CORES  # 512
K_TERMS = 10  # terms 0..9; max-rel error 8.6e-7 vs fp32 reference
F32 = mybir.dt.float32
MULT = mybir.AluOpType.mult


def _hoist_waits(nc: bass.Bass) -> int:
    """Move semaphore waits off instructions onto standalone EventSemaphore
    instructions. This walrus build rejects any wait attached to a Matmult
    (S3_LW struct) and allows at most one elsewhere ("Too many sync wait
    commands"); a preceding same-engine wait instruction is equivalent."""
    n = 0
    for f in nc.m.functions:
        for blk in f.blocks:
            il = blk.instructions
            i = 0
            while i < len(il):
                ins = il[i]
                si = ins.sync_info
                if si is None or not si.on_wait:
                    i += 1
                    continue
                keep = 0 if ins.__class__.__name__ in ("InstMatmult", "InstMatmultMx") else 1
                waits = list(si.on_wait)
                if len(waits) <= keep:
                    i += 1
                    continue
                hoisted = waits[: len(waits) - keep]
                si.on_wait = waits[len(waits) - keep:]
                for w in hoisted:
                    wi = mybir.InstEventSemaphore(
                        name=f"W-hoist-{n}", engine=ins.engine, ins=[], outs=[])
                    wi.sync_info = type(si)(on_wait=[w], on_update=[])
                    il.insert(i, wi)
                    n += 1
                    i += 1
                i += 1
    return n


def _build_program(hoist: bool = True) -> bass.Bass:
    nc = bass.Bass("TRN2", target_bir_lowering=False, debug=False,
                   enable_asserts=False, num_devices=N_CORES,
                   enable_partition_id=False)

    # aux packs identity | ones | W so one DMA covers all [128, .] inputs;
    # tb packs t (as a row) | bias on partition 0.
    x_d = nc.dram_tensor("x", [B_LOC, D], F32, kind="ExternalInput").ap()
    aux_d = nc.dram_tensor("aux", [D, 3 * D], F32, kind="ExternalInput").ap()
    tb_d = nc.dram_tensor("tb", [1, B_LOC + D], F32, kind="ExternalInput").ap()
    y_d = nc.dram_tensor("y", [B_LOC, D], F32, kind="ExternalOutput").ap()
    ljd_d = nc.dram_tensor("ljd", [B_LOC, D], F32, kind="ExternalOutput").ap()

    NT = B_LOC // D  # 4 batch tiles of 128
    HALF = B_LOC // 2  # 256: chain runs as two independent column-halves

    with tile.TileContext(nc) as tc, ExitStack() as ctx:
        const = ctx.enter_context(tc.tile_pool(name="const", bufs=1))
        upool = ctx.enter_context(tc.tile_pool(name="u", bufs=6))
        ps_sm = ctx.enter_context(tc.tile_pool(name="ps_sm", bufs=2, space="PSUM"))
        ps_t = ctx.enter_context(tc.tile_pool(name="ps_t", bufs=1, space="PSUM"))
        ps_chain = ctx.enter_context(tc.tile_pool(name="ps_chain", bufs=3, space="PSUM"))
        ps_out = ctx.enter_context(tc.tile_pool(name="ps_out", bufs=2, space="PSUM"))

        # ---- loads (three DMAs, issued from different engines so the
        # per-queue trigger cost overlaps) ----
        x_bm = const.tile([D, NT, D], F32, tag="x_bm")
        nc.sync.dma_start(x_bm, x_d.rearrange("(m p) i -> p m i", p=D))
        aux = const.tile([D, 3 * D], F32, tag="aux")
        nc.scalar.dma_start(aux, aux_d)
        ident = aux[:, 0:D]
        ones_col = aux[:, D:D + 1]
        ones_row = aux[0:1, D:2 * D]
        w_sb = aux[:, 2 * D:3 * D]
        tb = const.tile([1, B_LOC + D], F32, tag="tb")
        nc.gpsimd.dma_start(tb, tb_d)
        t_row = tb[:, 0:B_LOC]
        bias_row = tb[:, B_LOC:B_LOC + D]

        # ---- layout transposes: XT = x^T (feature-major), WT = W^T ----
        xt = const.tile([D, B_LOC], F32, tag="xt")
        for m in range(NT):
            ps = ps_sm.tile([D, D], F32, tag="ps_sm")
            nc.tensor.transpose(ps, x_bm[:, m, :], ident)
            nc.scalar.copy(xt[:, bass.ts(m, D)], ps)
        wt = const.tile([D, D], F32, tag="wt")
        ps = ps_sm.tile([D, D], F32, tag="ps_sm")
        nc.tensor.transpose(ps, w_sb, ident)
        nc.scalar.copy(wt, ps)

        # ---- T_rep[i, b] = t_b (broadcast across partitions, rank-1 matmul)
        t_rep = const.tile([D, B_LOC], F32, tag="t_rep")
        psT = ps_t.tile([D, B_LOC], F32, tag="psT")
        nc.tensor.matmul(psT, ones_row, t_row)
        nc.scalar.copy(t_rep, psT)

        # ---- diag(W) as a row: ones_col^T @ (W .* I) ----
        wi = const.tile([D, D], F32, tag="wi")
        nc.gpsimd.tensor_mul(wi, w_sb, ident)
        ps = ps_sm.tile([D, D], F32, tag="ps_sm")
        nc.tensor.matmul(ps[0:1, :], ones_col, wi)
        diag_row = const.tile([1, D], F32, tag="diag_row")
        nc.scalar.copy(diag_row, ps[0:1, :])

        # ---- ljd = t x diag(W), early: fills PE while the chain waits ----
        lo_all = const.tile([D, NT, D], F32, tag="lo_all")
        for m in range(NT):
            psl = ps_out.tile([D, D], F32, tag="ps_out")
            nc.tensor.matmul(psl, t_row[0:1, bass.ts(m, D)], diag_row)
            nc.scalar.copy(lo_all[:, m, :], psl)
        nc.scalar.dma_start(ljd_d.rearrange("(m p) i -> p m i", p=D), lo_all)

        # ---- Taylor chain. Y accumulates in SBUF; the per-term adds are
        # split across engines: half 0 on vector, half 1 on gpsimd. ----
        y_fm = const.tile([D, B_LOC], F32, tag="y_fm")
        nc.vector.tensor_copy(y_fm[:, 0:HALF], xt[:, 0:HALF])
        nc.gpsimd.tensor_copy(y_fm[:, HALF:B_LOC], xt[:, HALF:B_LOC])
        cur = [xt[:, 0:HALF], xt[:, HALF:B_LOC]]
        for k in range(1, K_TERMS):
            for h in range(2):
                sl = slice(h * HALF, (h + 1) * HALF)
                psc = ps_chain.tile([D, HALF], F32, tag="ps_chain")
                nc.tensor.matmul(psc, wt, cur[h])
                u_next = upool.tile([D, HALF], F32, tag="u")
                nc.vector.scalar_tensor_tensor(out=u_next, in0=psc,
                                               scalar=float(1.0 / k),
                                               in1=t_rep[:, sl],
                                               op0=MULT, op1=MULT)
                eng = nc.vector if h == 0 else nc.gpsimd
                eng.tensor_add(y_fm[:, sl], y_fm[:, sl], u_next)
                cur[h] = u_next[:]

        # ---- transpose Y to batch-major, accumulating bias x t in PSUM ----
        yo_all = const.tile([D, NT, D], F32, tag="yo_all")
        for m in range(NT):
            ps = ps_out.tile([D, D], F32, tag="ps_out")
            nc.tensor.transpose(ps, y_fm[:, bass.ts(m, D)], ident)
            nc.tensor.matmul(ps, t_row[0:1, bass.ts(m, D)], bias_row,
                             start=False, stop=True, skip_group_check=True)
            nc.scalar.copy(yo_all[:, m, :], ps)
        nc.sync.dma_start(y_d.rearrange("(m p) i -> p m i", p=D), yo_all)

    if hoist:
        _hoist_waits(nc)
    return nc


_CACHE: dict = {}


def _aux_np(w: np.ndarray) -> np.ndarray:
    c = np.zeros((D, 3 * D), dtype=np.float32)
    c[:, :D] = np.eye(D, dtype=np.float32)
    c[:, D:2 * D] = 1.0
    c[:, 2 * D:] = w
    return c


def _run(x, t, weight, bias, trace=False, **trace_kw):
    if "nc" not in _CACHE:
        _CACHE["nc"] = _build_program()
    nc = _CACHE["nc"]
    x = np.ascontiguousarray(x, dtype=np.float32)
    t = np.asarray(t, dtype=np.float32).reshape(B)
    w = np.asarray(weight, dtype=np.float32)
    bias = np.asarray(bias, dtype=np.float32).reshape(D)
    aux = _aux_np(w)
    in_maps = []
    for i in range(N_CORES):
        tb = np.concatenate([t[i * B_LOC:(i + 1) * B_LOC], bias])[None, :]
        in_maps.append({"x": x[i * B_LOC:(i + 1) * B_LOC],
                        "tb": np.ascontiguousarray(tb), "aux": aux})
    res = run_bass_kernel_spmd(nc, in_maps, list(range(N_CORES)),
                               trace=trace, **trace_kw)
    y = np.concatenate([res.results[i]["y"] for i in range(N_CORES)], axis=0)
    ljd = np.concatenate([res.results[i]["ljd"] for i in range(N_CORES)], axis=0)
    return (y, ljd), res


def kernel(x, t, weight, bias):
    (y, ljd), _ = _run(x, t, weight, bias, trace=False)
    return y, ljd
